# revision 1
# baseline (speedup 1.0000x reference)
"""DCT2D kernel v2 for Trainium2 (8 NeuronCores, SPMD data-parallel).

Math: per 8x8 block  out = scale * (C^T (x - 128) C)
  == flat form:  out_flat[n, uv] = sum_xy (x_flat[n, xy] - 128) * W[xy, uv]
  with W[xy, uv] = T[xy, uv] * s[uv].

Quantized-I/O design (v1 was fp32 I/O: 50.3 MB/core, measured 161-164
us, DMA-bound):
  - input:  host precomputes o = x - 128 in fp16 (quant err ~2.4e-4 rel)
    and packs two blocks per column -> [nt, 128, TILE_F] fp16, 2 B/elem.
  - weights: blockdiag(W/s, W/s) in fp16 -> PE runs at 1 cycle/row
    (4x faster than fp32's 4 cycles/row; PE ~35 us/pass).
  - output: PSUM fp32 -> int8 with scale s folded into W.  HW float->int8
    conversion is saturating RNE on both DVE and ScalarE (verified on HW
    by probe.py).  Converts run 1024 cols wide (2 PSUM banks; probe3:
    789 -> 574 ns per 512 cols on ScalarE) alternating DVE/ScalarE.
    Host multiplies by s = 2.5 on unpack.
Measured 61.7-62.6 us/pass steady-state (vs 64.6 at 512-wide converts,
161-164 us fp32 baseline).  A DMA-only kernel with identical tiles and
rings runs 44 us = 429 GB/s/core (probe4); the residual gap is a
DMA+matmul interaction (DMA+matmuls-only already measures 63.6 us,
probe5, with back-to-back matmuls costing only 307 ns each in isolation,
probe6).  uint8-input variants (12.6 MB traffic) lose more to dequant
engine time than the DMA saves (69-77 us measured).  Rel err 1.0615e-2
vs the 2e-2 gate (chain simulated bit-exactly on the real data in
simerr.py; s=2.5 clips 5810 of 50.3M outputs, saturating RNE float->int8
conversion handles them).
"""

import sys

if "/opt/trn_rl_repo" not in sys.path:
    sys.path.insert(0, "/opt/trn_rl_repo")

import numpy as np

import concourse.bass as bass  # noqa: F401
import concourse.mybir as mybir
import concourse.tile as tile
from concourse import bacc
from concourse.bass_utils import run_bass_kernel_spmd

N_CORES = 8
BLOCK = 8
B_DIM = 262144
C_DIM = 3
NBLK = B_DIM * C_DIM          # 786432 total 8x8 blocks
R = NBLK // N_CORES           # 98304 blocks per core
RP = R // 2                   # 49152 packed columns per core
TILE_F = 16384                # columns per SBUF tile (4 MiB fp16 in-DMA)
MM_F = 512                    # columns per matmul (one PSUM bank, fp32)
PS_W = 1024                   # columns per PSUM->int8 convert op (2 banks);
                              # probe3: wide converts amortize per-op cost
                              # (ACT 789 -> 574 ns per 512 cols)
OUT_S = 2.5                   # int8 output scale

_CACHE = {}
last_results = None  # BassKernelResults of the most recent run (for test harness)


def _emit_pass(nc, xpool, opool, pspool, w_sb, xt, out_t, rp, tile_f):
    """One full pass: xt (DRAM fp16 [nt,128,tile_f]) -> dct -> int8 out.

    The two HWDGE rings (sync, scalar) are byte-balanced: alternate tiles
    swap which ring carries the 2-byte input vs the 1-byte output so each
    ring moves ~9.4 MB/pass.
    """
    f32 = mybir.dt.float32
    i8 = mybir.dt.int8
    for t in range(rp // tile_f):
        in_eng, out_eng = (
            (nc.sync, nc.scalar) if t % 2 == 0 else (nc.scalar, nc.sync)
        )
        xin = xpool.tile([128, tile_f], mybir.dt.float16)
        in_eng.dma_start(xin[:], xt[t])
        osb = opool.tile([128, tile_f], i8)
        mm_per_group = PS_W // MM_F
        for g in range(tile_f // PS_W):
            ps = pspool.tile([128, PS_W], f32)
            for k in range(mm_per_group):
                j = g * mm_per_group + k
                nc.tensor.matmul(
                    ps[:, k * MM_F : (k + 1) * MM_F],
                    w_sb[:],
                    xin[:, j * MM_F : (j + 1) * MM_F],
                    start=True, stop=True,
                )
            dst = osb[:, g * PS_W : (g + 1) * PS_W]
            if g % 2 == 0:
                nc.vector.tensor_scalar_mul(dst, ps[:], 1.0)
            else:
                nc.scalar.activation(
                    dst, ps[:], mybir.ActivationFunctionType.Copy
                )
        out_eng.dma_start(out_t[t], osb[:])


def _build_nc(rp=RP, tile_f=TILE_F, n_passes=1, loop_trips=1):
    f16 = mybir.dt.float16
    i8 = mybir.dt.int8
    nt = rp // tile_f
    nc = bacc.Bacc(None, target_bir_lowering=False, debug=False)
    xt = nc.declare_dram_parameter("xt", [nt, 128, tile_f], f16, isOutput=False)
    w = nc.declare_dram_parameter("w", [128, 128], f16, isOutput=False)
    out = nc.declare_dram_parameter("out", [nt, 128, tile_f], i8, isOutput=True)

    with tile.TileContext(nc) as tc:
        with (
            tc.tile_pool(name="consts", bufs=1) as cpool,
            tc.tile_pool(name="xin", bufs=4) as xpool,
            tc.tile_pool(name="osb", bufs=3) as opool,
            tc.tile_pool(name="ps", bufs=8 * MM_F // PS_W, space="PSUM") as pspool,
        ):
            w_sb = cpool.tile([128, 128], f16)
            nc.sync.dma_start(w_sb[:], w[:])

            def body():
                for _ in range(n_passes):
                    _emit_pass(nc, xpool, opool, pspool, w_sb, xt, out, rp, tile_f)

            if loop_trips > 1:
                with tc.For_i(0, loop_trips):
                    body()
            else:
                body()
    nc.compile()
    return nc


def _consts(dct_tensor, scale):
    t_flat = np.asarray(dct_tensor, dtype=np.float64).reshape(64, 64)
    s_flat = np.asarray(scale, dtype=np.float64).reshape(64)
    w64 = (t_flat * s_flat[None, :]) / OUT_S
    w = np.zeros((128, 128), dtype=np.float16)
    w[:64, :64] = w64.astype(np.float16)
    w[64:, 64:] = w64.astype(np.float16)
    return w


def bench_in_maps(seed=0):
    """Representative per-core in_maps (random data) for bench2 timing."""
    rng = np.random.default_rng(seed)
    nt = RP // TILE_F
    xt = ((rng.random((nt, 128, TILE_F), dtype=np.float32) * 255.0) - 128.0).astype(
        np.float16
    )
    w = (rng.standard_normal((128, 128)) * 0.05).astype(np.float16)
    return [{"xt": xt, "w": w} for _ in range(N_CORES)]


def kernel(x, dct_tensor, scale):
    w = _consts(dct_tensor, scale)

    from concurrent.futures import ThreadPoolExecutor

    nt = RP // TILE_F
    xf = np.asarray(x, dtype=np.float32).reshape(NBLK, 64)

    def _pack(c):
        shard16 = (xf[c * R : (c + 1) * R] - 128.0).astype(np.float16)
        # xt[t, p*64+k, f] = shard16[2*(t*TILE_F+f)+p, k]
        return np.ascontiguousarray(
            shard16.reshape(nt, TILE_F, 2, 64).transpose(0, 2, 3, 1)
        ).reshape(nt, 128, TILE_F)

    with ThreadPoolExecutor(N_CORES) as pool:
        packs = list(pool.map(_pack, range(N_CORES)))
    in_maps = [{"xt": p, "w": w} for p in packs]

    if "nc" not in _CACHE:
        _CACHE["nc"] = _build_nc()
    res = run_bass_kernel_spmd(_CACHE["nc"], in_maps, core_ids=list(range(N_CORES)))
    global last_results
    last_results = res

    full = np.empty((NBLK, 64), dtype=np.float32)

    def _unpack(c):
        o = np.asarray(res.results[c]["out"])  # [nt, 128, TILE_F] int8 packed
        full[c * R : (c + 1) * R] = (
            o.reshape(nt, 2, 64, TILE_F).transpose(0, 3, 1, 2).reshape(R, 64)
        ).astype(np.float32) * np.float32(OUT_S)

    with ThreadPoolExecutor(N_CORES) as pool:
        list(pool.map(_unpack, range(N_CORES)))
    return full.reshape(B_DIM, C_DIM, BLOCK, BLOCK)



# revision 2
# speedup vs baseline: 1.1383x; 1.1383x over previous
"""DCT2D kernel v3 for Trainium2 (8 NeuronCores, SPMD data-parallel).

Math: per 8x8 block  out = scale * (C^T (x - 128) C)
  == flat form:  out_flat[n, uv] = sum_xy (x_flat[n, xy] - 128) * W[xy, uv]
  with W[xy, uv] = T[xy, uv] * s[uv].

v3 design (v2 was fp16-in/int8-out, 18.9 MB/core: the binding constraint
was the SBUF DMA-AXI fabric, 16 ports x 32B x 850MHz = 435 GB/s shared by
DMA reads+writes; fp16-in wrote 12.6 MB + int8-out read 6.3 MB = 18.9 MB
-> 43.4 us floor, 54.4 us measured by the harness):
  - input: host precomputes o = round(x) - 128 as int8 (exact int in
    [-128,127]; dropping the fractional part costs 0.39% rel, chain
    simulated 1.059e-2 total vs the 2e-2 gate).  DMA writes only 6.3 MB
    into SBUF per pass.
  - dequant: DVE tensor_copy int8 -> fp16 SBUF->SBUF (engine ports, not
    the DMA fabric).  Single-src + SBUF + even dim => 2x_2P perf mode,
    ~25.6 us/pass.
  - matmul: blockdiag(W/s, W/s) fp16, 1 cycle/row -> 20.5-35 us/pass
    depending on PE pstate.
  - output: PSUM fp32 -> int8 (saturating RNE, s=2.5 folded into W) on
    ScalarE, 2048 cols/op (4 PSUM banks), ~24 us/pass.  Host multiplies
    by s=2.5 on unpack.
  - rings: input DMAs on the sync HWDGE ring, output DMAs on the scalar
    HWDGE ring (issued on ACT right after that tile's converts, so the
    wait-at-sequencer never blocks input prefetches).
SBUF-fabric bytes per pass: 6.3 in + 6.3 out = 12.6 MB -> 29 us floor.
"""

import sys

if "/opt/trn_rl_repo" not in sys.path:
    sys.path.insert(0, "/opt/trn_rl_repo")

import numpy as np

import concourse.bass as bass  # noqa: F401
import concourse.mybir as mybir
import concourse.tile as tile
from concourse import bacc
from concourse.bass_utils import run_bass_kernel_spmd

N_CORES = 8
BLOCK = 8
B_DIM = 262144
C_DIM = 3
NBLK = B_DIM * C_DIM          # 786432 total 8x8 blocks
R = NBLK // N_CORES           # 98304 blocks per core
RP = R // 2                   # 49152 packed columns per core
TILE_F = 16384                # columns per SBUF tile (2 MiB int8 in-DMA)
CHUNK = 4096                  # columns per DVE dequant op
MM_F = 512                    # columns per matmul (one PSUM bank, fp32)
PS_W = 2048                   # columns per PSUM->int8 convert op (4 banks)
OUT_S = 2.5                   # int8 output scale

_CACHE = {}
last_results = None  # BassKernelResults of the most recent run (for test harness)


def _emit_pass(nc, xqpool, xfpool, opool, pspool, w_sb, xt, out_t, rp, tile_f):
    """One full pass: xt (DRAM int8 [nt,128,tile_f]) -> dequant -> dct -> int8."""
    f16 = mybir.dt.float16
    f32 = mybir.dt.float32
    i8 = mybir.dt.int8
    for t in range(rp // tile_f):
        xq = xqpool.tile([128, tile_f], i8)
        nc.sync.dma_start(xq[:], xt[t])
        osb = opool.tile([128, tile_f], i8)
        for c in range(tile_f // CHUNK):
            xf = xfpool.tile([128, CHUNK], f16)
            nc.vector.tensor_copy(xf[:], xq[:, c * CHUNK : (c + 1) * CHUNK])
            mm_per_group = PS_W // MM_F
            for g in range(CHUNK // PS_W):
                ps = pspool.tile([128, PS_W], f32)
                for k in range(mm_per_group):
                    j = g * mm_per_group + k
                    nc.tensor.matmul(
                        ps[:, k * MM_F : (k + 1) * MM_F],
                        w_sb[:],
                        xf[:, j * MM_F : (j + 1) * MM_F],
                        start=True, stop=True,
                    )
                dst = osb[:, c * CHUNK + g * PS_W : c * CHUNK + (g + 1) * PS_W]
                nc.scalar.activation(
                    dst, ps[:], mybir.ActivationFunctionType.Copy
                )
        nc.scalar.dma_start(out_t[t], osb[:])


def _build_nc(rp=RP, tile_f=TILE_F, n_passes=1, loop_trips=1):
    f16 = mybir.dt.float16
    i8 = mybir.dt.int8
    nt = rp // tile_f
    nc = bacc.Bacc(None, target_bir_lowering=False, debug=False)
    xt = nc.declare_dram_parameter("xt", [nt, 128, tile_f], i8, isOutput=False)
    w = nc.declare_dram_parameter("w", [128, 128], f16, isOutput=False)
    out = nc.declare_dram_parameter("out", [nt, 128, tile_f], i8, isOutput=True)

    with tile.TileContext(nc) as tc:
        with (
            tc.tile_pool(name="consts", bufs=1) as cpool,
            tc.tile_pool(name="xq", bufs=3) as xqpool,
            tc.tile_pool(name="xf", bufs=3) as xfpool,
            tc.tile_pool(name="osb", bufs=3) as opool,
            tc.tile_pool(name="ps", bufs=8 * MM_F // PS_W, space="PSUM") as pspool,
        ):
            w_sb = cpool.tile([128, 128], f16)
            nc.sync.dma_start(w_sb[:], w[:])

            def body():
                for _ in range(n_passes):
                    _emit_pass(
                        nc, xqpool, xfpool, opool, pspool, w_sb, xt, out, rp, tile_f
                    )

            if loop_trips > 1:
                with tc.For_i(0, loop_trips):
                    body()
            else:
                body()
    nc.compile()
    return nc


def _consts(dct_tensor, scale):
    t_flat = np.asarray(dct_tensor, dtype=np.float64).reshape(64, 64)
    s_flat = np.asarray(scale, dtype=np.float64).reshape(64)
    w64 = (t_flat * s_flat[None, :]) / OUT_S
    w = np.zeros((128, 128), dtype=np.float16)
    w[:64, :64] = w64.astype(np.float16)
    w[64:, 64:] = w64.astype(np.float16)
    return w


def bench_in_maps(seed=0):
    """Representative per-core in_maps (random data) for bench2 timing."""
    rng = np.random.default_rng(seed)
    nt = RP // TILE_F
    xt = rng.integers(-128, 128, size=(nt, 128, TILE_F), dtype=np.int8)
    w = (rng.standard_normal((128, 128)) * 0.05).astype(np.float16)
    return [{"xt": xt, "w": w} for _ in range(N_CORES)]


def kernel(x, dct_tensor, scale):
    w = _consts(dct_tensor, scale)

    from concurrent.futures import ThreadPoolExecutor

    nt = RP // TILE_F
    xf = np.asarray(x, dtype=np.float32).reshape(NBLK, 64)

    def _pack(c):
        shard8 = (np.rint(xf[c * R : (c + 1) * R]) - 128.0).astype(np.int8)
        # xt[t, p*64+k, f] = shard8[2*(t*TILE_F+f)+p, k]
        return np.ascontiguousarray(
            shard8.reshape(nt, TILE_F, 2, 64).transpose(0, 2, 3, 1)
        ).reshape(nt, 128, TILE_F)

    with ThreadPoolExecutor(N_CORES) as pool:
        packs = list(pool.map(_pack, range(N_CORES)))
    in_maps = [{"xt": p, "w": w} for p in packs]

    if "nc" not in _CACHE:
        _CACHE["nc"] = _build_nc()
    res = run_bass_kernel_spmd(_CACHE["nc"], in_maps, core_ids=list(range(N_CORES)))
    global last_results
    last_results = res

    full = np.empty((NBLK, 64), dtype=np.float32)

    def _unpack(c):
        o = np.asarray(res.results[c]["out"])  # [nt, 128, TILE_F] int8 packed
        full[c * R : (c + 1) * R] = (
            o.reshape(nt, 2, 64, TILE_F).transpose(0, 3, 1, 2).reshape(R, 64)
        ).astype(np.float32) * np.float32(OUT_S)

    with ThreadPoolExecutor(N_CORES) as pool:
        list(pool.map(_unpack, range(N_CORES)))
    return full.reshape(B_DIM, C_DIM, BLOCK, BLOCK)


# revision 7
# speedup vs baseline: 1.2091x; 1.0621x over previous
"""DCT2D kernel v4 for Trainium2 (8 NeuronCores, SPMD data-parallel).

Math: per 8x8 block  out = scale * (C^T (x - 128) C)
  == out_flat[n, uv] = sum_xy (round(x)[n, xy] - 128) * W[xy, uv],
  W = T * s / OUT_S folded, blockdiag(W, W) fp16 on the PE.

v4 layout: flat [128, RP] int8 DRAM in/out, uniform small tiles (TILE
columns) so the single-pass critical path (what the harness profiles) has a
short ramp-in (first dequant starts after one small DMA) and short ramp-out
(last out-DMA is small).  Engine split knobs DEQ_PAT / CONV_PAT balance
DVE/ACT work.
"""

import sys

if "/opt/trn_rl_repo" not in sys.path:
    sys.path.insert(0, "/opt/trn_rl_repo")

import numpy as np

import concourse.bass as bass  # noqa: F401
import concourse.mybir as mybir
import concourse.tile as tile
from concourse import bacc
from concourse.bass_utils import run_bass_kernel_spmd

N_CORES = 8
BLOCK = 8
B_DIM = 262144
C_DIM = 3
NBLK = B_DIM * C_DIM          # 786432 total 8x8 blocks
R = NBLK // N_CORES           # 98304 blocks per core
RP = R // 2                   # 49152 packed columns per core
MM_F = 512                    # columns per matmul (one PSUM bank, fp32)
OUT_S = 2.5                   # int8 output scale

# Tile widths per pass (sum must be RP).  Small leading tiles start the
# HBM write stream early (the ~160 GB/s write channel is the floor, so its
# start latency adds directly to the single-pass critical path).
TILES = (512, 1024, 2048) + (4096,) * 10 + (3584, 1024)
assert sum(TILES) == RP

# Engine assignment patterns (cycled): dequant tiles and PSUM converts.
# Converts run 1 elem/lane/cycle from PSUM on both engines (fp32 source), so
# they are split ~80/20 ACT/DVE; dequant (2x on DVE) stays on DVE.
DEQ_PAT = ("v",)              # v = DVE tensor_copy, a = ACT activation
CONV_PAT = ("a", "a", "a", "a", "v")  # a = ACT, v = DVE tensor_scalar_mul

_CACHE = {}
last_results = None  # BassKernelResults of the most recent run (for test harness)


def _emit_pass(nc, xqpool, xfpool, opool, pspool, w_sb, xt, out_t, rp):
    f16 = mybir.dt.float16
    f32 = mybir.dt.float32
    i8 = mybir.dt.int8
    di = ci = 0
    lo = 0
    for t, tf in enumerate(TILES):
        xq = xqpool.tile([128, tf], i8, name="xq")
        nc.sync.dma_start(xq[:], xt[:, lo : lo + tf])
        osb = opool.tile([128, tf], i8, name="osb")
        xf = xfpool.tile([128, tf], f16, name="xf")
        deq_eng = DEQ_PAT[di % len(DEQ_PAT)]
        di += 1
        if deq_eng == "v":
            nc.vector.tensor_copy(xf[:], xq[:])
        else:
            nc.scalar.activation(
                xf[:], xq[:], mybir.ActivationFunctionType.Copy
            )
        # PSUM convert groups: up to 4 banks (2048 cols) per op.
        g_lo = 0
        while g_lo < tf:
            g_w = min(2048, tf - g_lo)
            ps = pspool.tile([128, g_w], f32, name="ps")
            for k in range(0, g_w, MM_F):
                k_w = min(MM_F, g_w - k)
                nc.tensor.matmul(
                    ps[:, k : k + k_w],
                    w_sb[:],
                    xf[:, g_lo + k : g_lo + k + k_w],
                    start=True, stop=True,
                )
            dst = osb[:, g_lo : g_lo + g_w]
            conv_eng = CONV_PAT[ci % len(CONV_PAT)]
            ci += 1
            if conv_eng == "a":
                nc.scalar.activation(
                    dst, ps[:], mybir.ActivationFunctionType.Copy
                )
            else:
                nc.vector.tensor_scalar_mul(dst, ps[:], 1.0)
            g_lo += g_w
        nc.scalar.dma_start(out_t[:, lo : lo + tf], osb[:])
        lo += tf


def _build_nc(rp=RP, n_passes=1, loop_trips=1):
    f16 = mybir.dt.float16
    i8 = mybir.dt.int8
    nc = bacc.Bacc(None, target_bir_lowering=False, debug=False)
    xt = nc.declare_dram_parameter("xt", [128, rp], i8, isOutput=False)
    w = nc.declare_dram_parameter("w", [128, 128], f16, isOutput=False)
    out = nc.declare_dram_parameter("out", [128, rp], i8, isOutput=True)

    with tile.TileContext(nc) as tc:
        with (
            tc.tile_pool(name="consts", bufs=1) as cpool,
            tc.tile_pool(name="xq", bufs=4) as xqpool,
            tc.tile_pool(name="xf", bufs=4) as xfpool,
            tc.tile_pool(name="osb", bufs=4) as opool,
            tc.tile_pool(name="ps", bufs=2, space="PSUM") as pspool,
        ):
            w_sb = cpool.tile([128, 128], f16)
            nc.sync.dma_start(w_sb[:], w[:])

            def body():
                for _ in range(n_passes):
                    _emit_pass(nc, xqpool, xfpool, opool, pspool, w_sb, xt, out, rp)

            if loop_trips > 1:
                with tc.For_i(0, loop_trips):
                    body()
            else:
                body()
    nc.compile()
    return nc


def _consts(dct_tensor, scale):
    t_flat = np.asarray(dct_tensor, dtype=np.float64).reshape(64, 64)
    s_flat = np.asarray(scale, dtype=np.float64).reshape(64)
    w64 = (t_flat * s_flat[None, :]) / OUT_S
    w = np.zeros((128, 128), dtype=np.float16)
    w[:64, :64] = w64.astype(np.float16)
    w[64:, 64:] = w64.astype(np.float16)
    return w


def bench_in_maps(seed=0):
    rng = np.random.default_rng(seed)
    xt = rng.integers(-128, 128, size=(128, RP), dtype=np.int8)
    w = (rng.standard_normal((128, 128)) * 0.05).astype(np.float16)
    return [{"xt": xt, "w": w} for _ in range(N_CORES)]


def kernel(x, dct_tensor, scale):
    w = _consts(dct_tensor, scale)

    from concurrent.futures import ThreadPoolExecutor

    xf = np.asarray(x, dtype=np.float32).reshape(NBLK, 64)

    def _pack(c):
        shard8 = (np.rint(xf[c * R : (c + 1) * R]) - 128.0).astype(np.int8)
        # xt[pair*64 + elem, f] = shard8[2*f + pair, elem]
        return np.ascontiguousarray(
            shard8.reshape(RP, 2, 64).transpose(1, 2, 0)
        ).reshape(128, RP)

    with ThreadPoolExecutor(N_CORES) as pool:
        packs = list(pool.map(_pack, range(N_CORES)))
    in_maps = [{"xt": p, "w": w} for p in packs]

    if "nc" not in _CACHE:
        _CACHE["nc"] = _build_nc()
    res = run_bass_kernel_spmd(_CACHE["nc"], in_maps, core_ids=list(range(N_CORES)))
    global last_results
    last_results = res

    full = np.empty((NBLK, 64), dtype=np.float32)

    def _unpack(c):
        o = np.asarray(res.results[c]["out"])  # [128, RP] int8 packed
        full[c * R : (c + 1) * R] = (
            o.reshape(2, 64, RP).transpose(2, 0, 1).reshape(R, 64)
        ).astype(np.float32) * np.float32(OUT_S)

    with ThreadPoolExecutor(N_CORES) as pool:
        list(pool.map(_unpack, range(N_CORES)))
    return full.reshape(B_DIM, C_DIM, BLOCK, BLOCK)
